# revision 15
# baseline (speedup 1.0000x reference)
"""Distributed kNN (DkNN conformal credibility) on 8 TRN2 NeuronCores.

Math: the reference's per-layer normalize+center cancels for ranking ---
top-75 by EuclideanSquared of normalized-centered vectors == top-75 by
descending q . (t_n/||t_n||).  The host pre-scales the training columns by
their inverse norms, so the device is a pure scan over score strips.

v2 matmul scheme ("fp22 + cross"): the PE's fp32r mode truncates operands
to FP22 (e10m11).  The host pre-rounds T = 4096*that and q to 11 mantissa
bits so the truncation is exact, then one f32r pass computes q22.T22
exactly.  Two cheap correction passes restore ~21-bit effective precision:
a bf16 pass ql.bf16(T22) (the bf16 operand is a strided alias of the SAME
f32 bytes -- the top 2 bytes of an f32 are its bf16-RTZ), and an fp8
DoubleRow pass q8.t8 where t8 = e4m3(T - T22) rides in byte0 of the f32
(f32r ignores the low 12 mantissa bits; bits 11:8 stay zero so any
round-at-bit-12 also leaves T22 intact).  4 bytes/elem of DMA carry all
three operand views.  Scores come out scaled by 4096: ranking-invariant.

Labels are packed into the low 4 score mantissa bits, per-strip top-8 via
DVE max8, per-core top-32 peel, per-layer AllGather of the 8x32
candidates, and a replicated global top-75 -> class counts -> conformal
p-values via a host-precomputed searchsorted LUT -> argmax credibility.
A tiny dummy AllGather at kernel start absorbs cross-core launch skew so
the real collectives are fast; layer 0's gather/merge work is interleaved
into layer 1's strip loop (delayed a few strips so the collective's
latency never blocks the DVE stream).
"""

import os
import sys
import types

for _p in ("/opt/trn_rl_repo", "/root/.axon_site/_ro/trn_rl_repo"):
    if os.path.isdir(_p) and _p not in sys.path:
        sys.path.insert(0, _p)

import numpy as np

import concourse.bass as bass
import concourse.mybir as mybir
from concourse.tile import TileContext
from concourse.vector_clock import ScopedClock
from concourse.bass_utils import run_bass_kernel_spmd

# ---------------------------------------------------------------- constants
N_CORES = 8
L = 2
N_TRAIN = 100000
N_SHARD = N_TRAIN // N_CORES          # 12500
D = 512
B = 256
K = 75
C = 10
NB_CALI = 750
KD = D // 128                         # 4 contraction k-tiles
STRIP = 500                           # n-columns per strip (25 exact strips)
N_STRIPS = N_SHARD // STRIP           # 25
N_CAND = N_STRIPS * 8                 # 200 stage-1 candidates per (layer,rg)
N_ROUNDS = 3                          # local peel rounds of 8 -> top-24
N_KEEP = N_ROUNDS * 8                 # 24 shipped per (layer,rg,row)
K_MERGE_ROUNDS = 10                   # global peel -> top-80 >= 75
NEG_INF = -3.0e38
RG = B // 128                         # 2 query row-groups
VMAX = L * K + 1                      # 151 possible nonconformity values
TAIL_DELAY = 6                        # strips to wait before popping tail ops
STRIP8 = 512                          # fp8 residual tile padded: stride%16==0

F32 = mybir.dt.float32
F32R = mybir.dt.float32r
BF16 = mybir.dt.bfloat16
FP8 = mybir.dt.float8e4
U32 = mybir.dt.uint32

MODE = os.environ.get("KNN_MODE", "v2")       # "v2" | "v2nodr" | "bf16x3"
PACK = os.environ.get("KNN_PACK", "dve")   # "dve" | "gpsimd"


# ------------------------------------------------- tile tail-drain workaround
def _patched_drain_and_barrier(self, tick_clock, wait_clock):
    # walrus rejects >few sync waits on one instruction; the stock tail
    # drain aggregates every live semaphore.  Spread them over drains.
    nc = self.nc
    drain_inst = nc.sync.drain()
    wait_clock.add_sem_waits(
        drain_inst.ins, ScopedClock({None: tick_clock.global_clock})
    )
    si = drain_inst.ins.sync_info
    waits = list(si.on_wait) if si is not None else []
    if len(waits) > 1:
        si.on_wait = waits[:1]
        SyncInfo = type(si)
        for w in waits[1:]:
            wi = nc.sync.drain()
            wi.ins.sync_info = SyncInfo(on_wait=[w], on_update=[])

    nc.all_engine_barrier()
    assert self.sems is not None
    popped = nc._tile_sem_poison_stack.pop()
    assert popped is self._sem_poison
    nc.clear_and_free_semaphores(list(self.sems.allocated().values()))
    nc.all_engine_barrier()


TileContext._drain_and_barrier = _patched_drain_and_barrier

# Cap embedded sync-waits per instruction; hoist the excess onto injected
# same-engine NOPs placed immediately before (program order on one engine
# makes this equivalent).
_MAX_WAITS = 1
_orig_lower_ordered_insts = TileContext._lower_ordered_insts


def _redistribute_waits(self, ordered):
    nc = self.nc
    SyncInfo = None
    for bb_name, insts in ordered.items():
        new_list = []
        for inst in insts:
            si = getattr(inst, 'sync_info', None)
            waits = list(si.on_wait) if si is not None else []
            cap = 1 if not isinstance(inst, mybir.InstNoOp) else _MAX_WAITS
            if len(waits) > cap:
                if SyncInfo is None:
                    SyncInfo = type(si)
                keep = waits[:cap]
                extra = waits[cap:]
                si.on_wait = keep
                for j in range(0, len(extra), _MAX_WAITS):
                    nop = mybir.InstNoOp(
                        name=f"waitnop-{nc.next_id()}", ins=[], outs=[],
                        engine=inst.engine,
                    )
                    nop.sync_info = SyncInfo(
                        on_wait=extra[j:j + _MAX_WAITS], on_update=[]
                    )
                    nc.register_instruction(nop, overwrite=True)
                    new_list.append(nop)
            new_list.append(inst)
        insts[:] = new_list
    return _orig_lower_ordered_insts(self, ordered)


TileContext._lower_ordered_insts = _redistribute_waits


def _register_ntff_hook():
    """Optional: make run_bass_kernel_spmd(trace=True) work under axon."""
    if 'antenv.axon_hooks' not in sys.modules:
        m = types.ModuleType('antenv.axon_hooks')
        hook = [None]
        m.set_axon_ntff_profile_hook = lambda h: hook.__setitem__(0, h)
        m.get_axon_ntff_profile_hook = lambda: hook[0]
        sys.modules['antenv.axon_hooks'] = m
        try:
            import antenv
            antenv.axon_hooks = m
        except ImportError:
            pass
    try:
        from antenv.axon_hooks import set_axon_ntff_profile_hook
        from trn_agent_boot.trn_boot import _ntff_profile_via_ctypes
        h = _ntff_profile_via_ctypes('/opt/axon/libaxon_pjrt.so')
        if h is not None:
            set_axon_ntff_profile_hook(h)
    except Exception:
        pass


_register_ntff_hook()


# ------------------------------------------------------------- kernel build
def build_kernel():
    nc = bass.Bass("TRN2", num_devices=N_CORES)

    # 5B/elem train stream: f32 = RN22(4096*that) + separate e4m3 residual
    tS = nc.declare_dram_parameter(
        "tS", [L, N_STRIPS, 128, KD, STRIP], F32R, isOutput=False)
    t8S = nc.declare_dram_parameter(
        "t8S", [L, N_STRIPS, 128, KD, STRIP8], FP8, isOutput=False)
    qS = nc.declare_dram_parameter("qS", [L, 128, KD, B], F32R, isOutput=False)
    qL = nc.declare_dram_parameter("qL", [L, 128, KD, B], BF16, isOutput=False)
    q8 = nc.declare_dram_parameter("q8", [L, 128, KD, B], FP8, isOutput=False)
    labels = nc.declare_dram_parameter("labels", [N_SHARD], U32, isOutput=False)
    luti = nc.declare_dram_parameter("luti", [2 * VMAX], F32, isOutput=False)
    tbrow = nc.declare_dram_parameter("tbrow", [C], F32, isOutput=False)
    creds = nc.declare_dram_parameter("creds", [B, C], F32, isOutput=True)

    lc = [nc.dram_tensor(f"lc{l}", [128, RG * N_KEEP], F32) for l in range(L)]
    gs = [nc.dram_tensor(f"gs{l}", [N_CORES, 128, RG * N_KEEP], F32,
                         addr_space="Shared") for l in range(L)]
    # dummy skew-absorbing collective payload
    dumL = nc.dram_tensor("dumL", [128, 8], F32)
    dumG = nc.dram_tensor("dumG", [N_CORES, 128, 8], F32, addr_space="Shared")

    with TileContext(nc) as tc, \
         tc.tile_pool(name="persist", bufs=1) as persist, \
         tc.tile_pool(name="tin", bufs=8) as tin_pool, \
         tc.tile_pool(name="t8p", bufs=8) as t8_pool, \
         tc.tile_pool(name="pk", bufs=8) as pk_pool, \
         tc.tile_pool(name="pss", bufs=8, space="PSUM") as pss_pool:

        # ---------------- setup
        # query weights ride the scalar hwdge queue: the sync queue stays
        # clear for strip DMAs so the PE stream starts immediately
        qw, qlw, q8w = [], [], []
        for l in range(L):
            qt = persist.tile([128, KD, B], F32R, name=f"qw{l}", tag=f"qw{l}")
            nc.scalar.dma_start(out=qt[:], in_=qS[l])
            qw.append(qt)
            qlt = persist.tile([128, KD, B], BF16, name=f"qlw{l}", tag=f"qlw{l}")
            nc.gpsimd.dma_start(out=qlt[:], in_=qL[l])
            qlw.append(qlt)
            q8t = persist.tile([128, KD, B], FP8, name=f"q8w{l}", tag=f"q8w{l}")
            nc.gpsimd.dma_start(out=q8t[:], in_=q8[l])
            q8w.append(q8t)

        mask_hi = persist.tile([128, 1], U32)
        nc.vector.memset(mask_hi[:], 0xFFFFFFF0)
        mask_lo = persist.tile([128, 1], U32, name="mask_lo", tag="mask_lo")
        nc.vector.memset(mask_lo[:], 0xF)

        # dummy early AllGather: aligns the 8 cores right after launch so
        # the real collectives don't absorb the cross-core skew
        dumt = persist.tile([128, 8], F32, name="dumt", tag="dumt")
        nc.vector.memset(dumt[:], 0.0)
        nc.gpsimd.dma_start(out=dumL[:], in_=dumt[:])
        nc.gpsimd.collective_compute(
            "AllGather",
            mybir.AluOpType.bypass,
            replica_groups=[list(range(N_CORES))],
            ins=[dumL[:]],
            outs=[dumG[:]],
        )

        # broadcasts ride the scalar hwdge queue so the sync queue's strip
        # stream starts immediately; label chunks unblock early strips fast
        labb = persist.tile([128, N_SHARD], U32, name="labb", tag="labb")
        LCH = STRIP

        def labb_chunk(idx):
            c0 = idx * LCH
            nc.scalar.dma_start(
                out=labb[:, c0:c0 + LCH],
                in_=labels[c0:c0 + LCH].partition_broadcast(128),
            )
        lutb = persist.tile([128, 2 * VMAX], F32, name="lutb", tag="lutb")
        iot = lutb[:, 0:VMAX]
        lut = lutb[:, VMAX:2 * VMAX]
        tbb = persist.tile([128, C], F32, name="tbb", tag="tbb")

        def lut_dmas():
            nc.scalar.dma_start(out=lutb[:],
                                in_=luti[:].partition_broadcast(128))
            nc.scalar.dma_start(out=tbb[:],
                                in_=tbrow[:].partition_broadcast(128))

        cands = [
            [persist.tile([128, N_CAND], F32, name=f"cand{l}_{rg}",
                          tag=f"cand{l}_{rg}") for rg in range(RG)]
            for l in range(L)
        ]
        wins = [
            persist.tile([128, RG * N_KEEP], F32, name=f"win{l}",
                         tag=f"win{l}") for l in range(L)
        ]
        gcands = [
            [persist.tile([128, N_CORES * N_KEEP], F32, name=f"gc{l}_{rg}",
                          tag=f"gc{l}_{rg}") for rg in range(RG)]
            for l in range(L)
        ]
        gwins = [
            [persist.tile([128, K_MERGE_ROUNDS * 8], F32, name=f"gw{l}_{rg}",
                          tag=f"gw{l}_{rg}") for rg in range(RG)]
            for l in range(L)
        ]
        labws = [
            [persist.tile([128, K], U32, name=f"lw{l}_{rg}",
                          tag=f"lw{l}_{rg}") for rg in range(RG)]
            for l in range(L)
        ]
        labwfs = [
            [persist.tile([128, K], F32, name=f"lwf{l}_{rg}",
                          tag=f"lwf{l}_{rg}") for rg in range(RG)]
            for l in range(L)
        ]
        cnts = [
            [persist.tile([128, C], F32, name=f"cnt{l}_{rg}",
                          tag=f"cnt{l}_{rg}") for rg in range(RG)]
            for l in range(L)
        ]
        scr = persist.tile([128, VMAX], F32, name="scr", tag="scr")

        # ---------------- tail-phase op factories (emitted interleaved)
        def peel(l, rg):
            # local top-24 peel; one combined AllGather per layer after rg1
            cd = cands[l][rg]
            for r in range(N_ROUNDS):
                w8 = wins[l][:, rg * N_KEEP + r * 8:rg * N_KEEP + (r + 1) * 8]
                nc.vector.max(out=w8, in_=cd[:])
                if r < N_ROUNDS - 1:
                    nc.vector.match_replace(
                        out=cd[:], in_to_replace=w8,
                        in_values=cd[:], imm_value=NEG_INF,
                    )
            if rg == RG - 1:
                nc.gpsimd.dma_start(out=lc[l][:], in_=wins[l][:])
                nc.gpsimd.collective_compute(
                    "AllGather",
                    mybir.AluOpType.bypass,
                    replica_groups=[list(range(N_CORES))],
                    ins=[lc[l][:]],
                    outs=[gs[l][:]],
                )

        def gcand_dma_op(l, rg):
            def run():
                nc.gpsimd.dma_start(
                    out=gcands[l][rg][:].rearrange("p (c k) -> p c k", c=N_CORES),
                    in_=gs[l][:, :, rg * N_KEEP:(rg + 1) * N_KEEP]
                        .rearrange("c p k -> p c k"),
                )
            return run

        def merge_round_op(l, rg, r):
            def run():
                gcand, gwin = gcands[l][rg], gwins[l][rg]
                w8 = gwin[:, r * 8:(r + 1) * 8]
                nc.vector.max(out=w8, in_=gcand[:])
                if r < K_MERGE_ROUNDS - 1:
                    nc.vector.match_replace(
                        out=gcand[:], in_to_replace=w8,
                        in_values=gcand[:], imm_value=NEG_INF,
                    )
            return run

        def counts_op(l, rg):
            def run():
                labw, labwf = labws[l][rg], labwfs[l][rg]
                nc.vector.tensor_scalar(
                    out=labw[:], in0=gwins[l][rg][:, 0:K].bitcast(U32),
                    scalar1=mask_lo[:], scalar2=None,
                    op0=mybir.AluOpType.bitwise_and,
                )
                nc.vector.tensor_copy(labwf[:], labw[:])
                cnt = cnts[l][rg]
                for c in range(C):
                    nc.vector.tensor_scalar(
                        out=scr[:, 0:K], in0=labwf[:],
                        scalar1=float(c), scalar2=0.0,
                        op0=mybir.AluOpType.is_equal, op1=mybir.AluOpType.add,
                        accum_out=cnt[:, c:c + 1],
                    )
            return run

        def make_tail(l):
            ops = [gcand_dma_op(l, 0), gcand_dma_op(l, 1)]
            for r in range(K_MERGE_ROUNDS):
                ops.append(merge_round_op(l, 0, r))
            for r in range(K_MERGE_ROUNDS):
                ops.append(merge_round_op(l, 1, r))
            ops.append(counts_op(l, 0))
            ops.append(counts_op(l, 1))
            return ops

        # ---------------- main loop: per layer, per 500-column strip
        tail_ops = []
        for l in range(L):
            for s in range(N_STRIPS):
                n0 = s * STRIP
                tin = tin_pool.tile([128, KD, STRIP], F32R,
                                    name="tin", tag="tin")
                nc.sync.dma_start(out=tin[:, 0:2, :], in_=tS[l, s, :, 0:2, :])
                nc.gpsimd.dma_start(out=tin[:, 2:4, :], in_=tS[l, s, :, 2:4, :])
                t8t = t8_pool.tile([128, KD, STRIP8], FP8,
                                   name="t8t", tag="t8t")
                nc.scalar.dma_start(out=t8t[:], in_=t8S[l, s])
                if l == 0 and s < N_SHARD // LCH:
                    labb_chunk(s)
                if l == 1 and s == 0:
                    lut_dmas()
                # bf16 alias: top 2 bytes of each f32 = its bf16-RTZ
                tin_bf = tin[:].bitcast(BF16)    # [128, KD, 2*STRIP]

                for rg in range(RG):
                    bs = rg * 128
                    pss = pss_pool.tile([128, STRIP], F32)
                    # ordering sandwiches f32r between bf16 groups so f32r
                    # and fp8-DoubleRow are never adjacent (FP32-HI hazard)
                    # P2a: query-residual cross vs bf16 alias (k=0,1)
                    for k in range(2):
                        nc.tensor.matmul(
                            pss[:],
                            qlw[l][:, k, bs:bs + 128],
                            tin_bf[:, k, 1::2],
                            start=(k == 0), stop=False,
                            skip_group_check=True,
                        )
                    # P1: exact fp22 x fp22 (host pre-rounded both operands)
                    for k in range(KD):
                        nc.tensor.matmul(
                            pss[:],
                            qw[l][:, k, bs:bs + 128],
                            tin[:, k, :],
                            start=False, stop=False,
                            skip_group_check=True,
                        )
                    # P2b: remaining bf16 cross (k=2,3)
                    for k in range(2, KD):
                        nc.tensor.matmul(
                            pss[:],
                            qlw[l][:, k, bs:bs + 128],
                            tin_bf[:, k, 1::2],
                            start=False, stop=False,
                            skip_group_check=True,
                        )
                    if MODE == "v2":
                        # P3: train-residual cross, fp8 DoubleRow (2 k-pairs)
                        for kp in range(KD // 2):
                            nc.tensor.matmul(
                                pss[:],
                                q8w[l][:, 2 * kp:2 * kp + 2, bs:bs + 128],
                                t8t[:, 2 * kp:2 * kp + 2, 0:STRIP],
                                start=False, stop=(kp == KD // 2 - 1),
                                perf_mode=mybir.MatmulPerfMode.DoubleRow,
                                skip_group_check=True,
                            )
                    else:  # v2nodr: plain fp8 matmuls (slow but same math)
                        for k in range(KD):
                            nc.tensor.matmul(
                                pss[:],
                                q8w[l][:, k, bs:bs + 128],
                                t8t[:, k, 0:STRIP],
                                start=False, stop=(k == KD - 1),
                                skip_group_check=True,
                            )

                    pk = pk_pool.tile([128, STRIP], F32)
                    if PACK == "gpsimd":
                        # ACT evicts PSUM; GpSimd masks + ORs the labels in.
                        # Pool rejects AP scalars (TensorScalarPtr) and the
                        # API only emits f32 imms, so patch in a u32 imm.
                        nc.scalar.copy(pk[:], pss[:])
                        inst = nc.gpsimd.scalar_tensor_tensor(
                            out=pk[:].bitcast(U32), in0=pk[:].bitcast(U32),
                            scalar=0.0, in1=labb[:, n0:n0 + STRIP],
                            op0=mybir.AluOpType.bitwise_and,
                            op1=mybir.AluOpType.bitwise_or,
                        )
                        args = list(inst.ins.ins)
                        args[1] = mybir.ImmediateValue(
                            dtype=U32, value=0xFFFFFFF0)
                        inst.ins.ins = args
                    else:
                        # single DVE op: evict PSUM + mask + label-pack
                        nc.vector.scalar_tensor_tensor(
                            out=pk[:].bitcast(U32),
                            in0=pss[:].bitcast(U32),
                            scalar=mask_hi[:],
                            in1=labb[:, n0:n0 + STRIP],
                            op0=mybir.AluOpType.bitwise_and,
                            op1=mybir.AluOpType.bitwise_or,
                        )
                    cd = cands[l][rg]
                    nc.vector.max(out=cd[:, s * 8:s * 8 + 8], in_=pk[:])
                    if s == N_STRIPS - 1:
                        peel(l, rg)

                # interleave previous layer's gather/merge into this loop,
                # delayed so the AllGather latency never stalls the DVE
                if tail_ops and s >= TAIL_DELAY:
                    tail_ops.pop(0)()

            for op in tail_ops:      # drain any leftover interleaved ops
                op()
            tail_ops = make_tail(l)
        for op in tail_ops:
            op()

        # ---------------- final: conformal p-values + credibility
        for rg in range(RG):
            cnt = persist.tile([128, C], F32, name=f"cT{rg}", tag=f"cT{rg}")
            nc.vector.tensor_tensor(
                out=cnt[:], in0=cnts[0][rg][:], in1=cnts[1][rg][:],
                op=mybir.AluOpType.add,
            )
            # v = 150 - cnt
            nc.vector.tensor_scalar(
                out=cnt[:], in0=cnt[:], scalar1=-1.0, scalar2=float(L * K),
                op0=mybir.AluOpType.mult, op1=mybir.AluOpType.add,
            )
            # m = LUT[v] via one-hot dot against the iota row
            mge = persist.tile([128, C], F32, name=f"mge{rg}", tag=f"mge{rg}")
            for c in range(C):
                nc.vector.scalar_tensor_tensor(
                    out=scr[:], in0=iot, scalar=cnt[:, c:c + 1], in1=lut,
                    op0=mybir.AluOpType.is_equal, op1=mybir.AluOpType.mult,
                    accum_out=mge[:, c:c + 1],
                )
            # mp = m*16 + (15-c): argmax prefers larger m then smaller class
            mp = persist.tile([128, C], F32, name=f"mp{rg}", tag=f"mp{rg}")
            nc.vector.scalar_tensor_tensor(
                out=mp[:], in0=mge[:], scalar=16.0, in1=tbb[:],
                op0=mybir.AluOpType.mult, op1=mybir.AluOpType.add,
            )
            rmax = persist.tile([128, 1], F32, name=f"rmax{rg}", tag=f"rmax{rg}")
            nc.vector.tensor_reduce(
                out=rmax[:], in_=mp[:], axis=mybir.AxisListType.X,
                op=mybir.AluOpType.max,
            )
            mask = persist.tile([128, C], F32, name=f"mask{rg}", tag=f"mask{rg}")
            nc.vector.tensor_scalar(
                out=mask[:], in0=mp[:], scalar1=rmax[:], scalar2=None,
                op0=mybir.AluOpType.is_equal,
            )
            crd = persist.tile([128, C], F32, name=f"crd{rg}", tag=f"crd{rg}")
            nc.vector.scalar_tensor_tensor(
                out=crd[:], in0=mask[:], scalar=1.0 / NB_CALI, in1=mge[:],
                op0=mybir.AluOpType.mult, op1=mybir.AluOpType.mult,
            )
            nc.sync.dma_start(out=creds[rg * 128:(rg + 1) * 128, :], in_=crd[:])

    return nc


_CACHE = {}


def _rn22(x):
    """Round f32 to 11 stored mantissa bits (exactly representable in FP22)."""
    u = np.ascontiguousarray(x, dtype=np.float32).view(np.uint32)
    u2 = ((u.astype(np.uint64) + 0x800) & 0xFFFFF000).astype(np.uint32)
    return u2.view(np.float32)


def _strip_layout(tT):
    """[L, D, NS] -> [L, N_STRIPS, 128, KD, STRIP] with d = k*128 + p."""
    x = tT.reshape(tT.shape[0], KD, 128, N_STRIPS, STRIP)
    return np.ascontiguousarray(x.transpose(0, 3, 2, 1, 4))


def _q_layout(qT):
    """[L, D, B] -> [L, 128, KD, B] with d = k*128 + p."""
    x = qT.reshape(qT.shape[0], KD, 128, B)
    return np.ascontiguousarray(x.transpose(0, 2, 1, 3))


def _prep_inputs(train_feats, query_feats, train_labels, cali_nonconformity):
    import ml_dtypes
    train_feats = np.ascontiguousarray(train_feats, dtype=np.float32)
    query_feats = np.ascontiguousarray(query_feats, dtype=np.float32)
    labels = np.asarray(train_labels).astype(np.uint32)
    cali = np.asarray(cali_nonconformity).astype(np.int64)

    # host pre-scale: that = t / ||t|| (ranking-equivalent to the reference);
    # x4096 puts the fp8 residual and the main pass on one PSUM scale
    nrm = np.linalg.norm(train_feats, axis=-1, keepdims=True).astype(np.float32)
    T = (train_feats / nrm).astype(np.float32) * np.float32(4096.0)

    qT = np.ascontiguousarray(query_feats.transpose(0, 2, 1))   # [L, D, B]
    q22 = _rn22(qT)
    qlo = (qT - q22).astype(ml_dtypes.bfloat16)
    qf8 = qT.astype(ml_dtypes.float8_e4m3fn)

    # searchsorted LUT: v in [0,150] -> #{cali >= v} (= NB - bisect_left)
    vgrid = np.arange(VMAX, dtype=np.int64)
    mlut = (NB_CALI - np.searchsorted(cali, vgrid, side='left')).astype(np.float32)
    luti = np.concatenate([vgrid.astype(np.float32), mlut])
    tbrow = (15.0 - np.arange(C, dtype=np.float32))

    common = {
        "luti": luti, "tbrow": tbrow,
        "qS": _q_layout(q22),
        "qL": _q_layout(qlo),
        "q8": _q_layout(qf8),
    }

    in_maps = []
    for i in range(N_CORES):
        sl = slice(i * N_SHARD, (i + 1) * N_SHARD)
        tT = np.ascontiguousarray(T[:, sl, :].transpose(0, 2, 1))  # [L,D,NS]
        t22 = _rn22(tT)
        t8 = (tT - t22).astype(ml_dtypes.float8_e4m3fn)            # residual
        m = dict(common)
        m["tS"] = _strip_layout(t22)
        t8s = _strip_layout(t8)        # [L, N_STRIPS, 128, KD, STRIP]
        t8p = np.zeros(t8s.shape[:-1] + (STRIP8,), ml_dtypes.float8_e4m3fn)
        t8p[..., :STRIP] = t8s
        m["t8S"] = t8p
        m["labels"] = np.ascontiguousarray(labels[sl])
        in_maps.append(m)
    return in_maps


def kernel(train_feats, query_feats, train_labels, cali_nonconformity,
           trace=False, **trace_kwargs):
    if "nc" not in _CACHE:
        _CACHE["nc"] = build_kernel()
    nc = _CACHE["nc"]
    in_maps = _prep_inputs(
        train_feats, query_feats, train_labels, cali_nonconformity
    )
    res = run_bass_kernel_spmd(
        nc, in_maps, list(range(N_CORES)), trace=trace, **trace_kwargs
    )
    _CACHE["last_result"] = res
    return np.asarray(res.results[0]["creds"], dtype=np.float32)
